# revision 24
# baseline (speedup 1.0000x reference)
"""Trainium2 Bass kernel for YOLO-style DetectionLoss.

Contract: kernel(**inputs) takes the FULL inputs (batch 512) and returns the
full output (5-tuple of f32 scalars), sharding batch-wise across 8 NeuronCores.

Per-core device program (64 images, 2048 GTs):
  - stream the predictions shard (12.5 MB) through SBUF in 8 contiguous DMAs,
    accumulating sum(softplus(objectness)) over channels {0,5} of every cell
    (ACT exp -> ln(1+x) with accum_out)
  - compute GT cell row indices on-device, gather the 2048 GT cells with one
    indirect DMA
  - decode boxes (sigmoid via exp+reciprocal; min(exp,1)), pick the
    responsible box via cross-multiplied IoU comparison (no divide), and
    accumulate coord / obj / class losses
  - deduplicate cells holding >=1 GT (pairwise compare within each image via
    a partner-partition stream_shuffle) and subtract their softplus terms
    from the noobj sum
  - reduce all partials across partitions with a ones-vector matmul -> 12
    scalars, summed on host across the 8 cores.
"""
import sys

sys.path.insert(0, "/opt/trn_rl_repo")

import numpy as np

import concourse.bass as bass
import concourse.tile as tile
from concourse import bacc, mybir

S = 52
NBOX = 2
NCLS = 8
EPS = 1e-6
LAMBDA_COORD = 5.0
LAMBDA_NOOBJ = 0.5
BATCH = 512
N_GT = 32
NCORES = 8
NIMG = BATCH // NCORES          # 64 images per core
CELLS = S * S                   # 2704
ROWS = NIMG * CELLS             # 173056 rows of 18 floats per core
NG = NIMG * N_GT                # 2048 GTs per core
P = 128
JJ = NG // P                    # 16 GTs per partition
NCHUNK = 8                      # streaming chunks
FREE = NIMG * CELLS * 18 // NCHUNK // P   # 3042 f32 per partition per chunk
BIG = 1.0e7                     # invalid-GT row sentinel offset (exact in f32)

f32 = mybir.dt.float32
i32 = mybir.dt.int32
Act = mybir.ActivationFunctionType
Op = mybir.AluOpType
AxX = mybir.AxisListType.X

# consts layout (128, 288):
#   [0:16)    row_base   = (g // 32) * 2704, g = p*16+j
#   [16:24)   iota over classes 0..7
#   [24:280)  lower-strict-triangular mask tri[j*16+q] = 1.0 if q < j
#   [280]     parity = p % 2
#   [281]     ones
#   [282]     EPS (1e-6)
CONST_W = 288


def _build_consts() -> np.ndarray:
    c = np.zeros((P, CONST_W), np.float32)
    p = np.arange(P)[:, None]
    j = np.arange(JJ)[None, :]
    g = p * JJ + j
    c[:, 0:16] = (g // N_GT) * CELLS
    c[:, 16:24] = np.arange(NCLS)[None, :]
    tri = (np.arange(JJ)[None, :] < np.arange(JJ)[:, None]).astype(np.float32)
    c[:, 24:280] = tri.reshape(-1)[None, :]
    c[:, 280] = (np.arange(P) % 2).astype(np.float32)
    c[:, 281] = 1.0
    c[:, 282] = EPS
    return c


_ACT_PATCHED = False


def _force_single_act_table():
    """Make the act-table-load pass place every activation in
    natural_log_exp_and_others (covers Exp+Ln), so the kernel pays one
    ACT_TABLE_LOAD instead of thrashing between per-function sets.
    Indices of the table dict are preserved (walrus resolves the id
    against the same act_info.json)."""
    global _ACT_PATCHED
    if _ACT_PATCHED:
        return
    from concourse import hw_specs

    orig = hw_specs.get_activation_tables

    def patched(arch):
        t = orig(arch)
        keep = "natural_log_exp_and_others"
        if keep not in t:
            return t
        return {k: (v if k == keep else set()) for k, v in t.items()}

    hw_specs.get_activation_tables = patched
    bacc.get_activation_tables = patched
    _ACT_PATCHED = True


def build_program(for_sim: bool = False, debug: bool = False) -> bass.Bass:
    _force_single_act_table()
    nc = bacc.Bacc(None, target_bir_lowering=False)

    pred = nc.dram_tensor("pred", [ROWS, 18], f32, kind="ExternalInput")
    gtb_d = nc.dram_tensor("gtb", [P, JJ * 4], f32, kind="ExternalInput")
    gtl_d = nc.dram_tensor("gtl", [P, JJ], f32, kind="ExternalInput")
    gtv_d = nc.dram_tensor("gtv", [P, JJ], f32, kind="ExternalInput")
    cst_d = nc.dram_tensor("consts", [P, CONST_W], f32, kind="ExternalInput")
    out_d = nc.dram_tensor("out", [1, 4 + NCHUNK], f32, kind="ExternalOutput")
    if debug:
        dbg_d = nc.dram_tensor("dbg", [P, 12 * JJ], f32, kind="ExternalOutput")

    shuffle_mask = []
    for i in range(0, 32, 2):
        shuffle_mask += [i + 1, i]

    with tile.TileContext(nc) as tc:
        with (
            tc.tile_pool(name="main", bufs=1) as mp,
            tc.tile_pool(name="stream", bufs=8) as sp,
            tc.tile_pool(name="psum", bufs=1, space="PSUM") as pp,
        ):
            # ---- small input DMAs
            cst = mp.tile([P, CONST_W], f32)
            nc.gpsimd.dma_start(out=cst[:], in_=cst_d[:])
            gtb = mp.tile([P, JJ * 4], f32)
            nc.gpsimd.dma_start(out=gtb[:], in_=gtb_d[:])
            gtl = mp.tile([P, JJ], f32)
            nc.gpsimd.dma_start(out=gtl[:], in_=gtl_d[:])
            gtv = mp.tile([P, JJ], f32)
            nc.gpsimd.dma_start(out=gtv[:], in_=gtv_d[:])

            gtb3 = gtb[:].rearrange("p (j c) -> p j c", c=4)
            stats_dve = mp.tile([P, 4], f32)   # coord, obj, cls, corr
            stats_act = mp.tile([P, NCHUNK], f32)

            # ---- GT cell indices: gj from cx, gi from cy (layout (j, c):
            #      c=0 -> gj, c=1 -> gi)
            t52 = mp.tile([P, 2 * JJ], f32)
            t52v = t52[:].rearrange("p (j c) -> p j c", c=2)
            nc.vector.tensor_scalar(t52v, gtb3[:, :, 0:2], 52.0, None, Op.mult)
            # floor(x): int cast rounds-to-nearest on HW (truncates in sim);
            # r - (r > x) is exact floor under either behavior for x >= 0
            gjii = mp.tile([P, 2 * JJ], i32)
            nc.vector.tensor_copy(out=gjii[:], in_=t52[:])
            gjif = mp.tile([P, 2 * JJ], f32)
            nc.vector.tensor_copy(out=gjif[:], in_=gjii[:])
            gadj = mp.tile([P, 2 * JJ], f32)
            nc.vector.tensor_tensor(gadj[:], gjif[:], t52[:], op=Op.is_gt)
            gjfl = mp.tile([P, 2 * JJ], f32)
            nc.vector.tensor_tensor(gjfl[:], gjif[:], gadj[:], op=Op.subtract)
            gji = mp.tile([P, 2 * JJ], f32)
            nc.vector.tensor_scalar(gji[:], gjfl[:], 51.0, 0.0, Op.min, Op.max)
            gji3 = gji[:].rearrange("p (j c) -> p j c", c=2)

            # row = row_base + gi*52 + gj
            rowa = mp.tile([P, JJ], f32)
            nc.vector.scalar_tensor_tensor(
                out=rowa[:], in0=gji3[:, :, 1], scalar=52.0, in1=gji3[:, :, 0],
                op0=Op.mult, op1=Op.add)
            rowf = mp.tile([P, JJ], f32)
            nc.vector.tensor_tensor(
                out=rowf[:], in0=rowa[:], in1=cst[:, 0:16], op=Op.add)
            rowi = mp.tile([P, JJ], i32)
            nc.vector.tensor_copy(out=rowi[:], in_=rowf[:])

            # ---- streaming softplus over all objectness logits
            predv = pred[:].rearrange("(c p f) d -> c p (f d)", c=NCHUNK, p=P)
            for c in range(NCHUNK):
                st = sp.tile([P, FREE], f32, tag="st")
                nc.sync.dma_start(out=st[:], in_=predv[c])
                st3 = st[:].rearrange("p (f d) -> p f d", d=18)
                e = sp.tile([P, FREE // 9], f32, tag="spe")
                e3 = e[:].rearrange("p (f d) -> p f d", d=2)
                nc.scalar.activation(out=e3, in_=st3[:, :, 0:10:5], func=Act.Exp)
                sl = sp.tile([P, FREE // 9], f32, tag="spl")
                nc.scalar.activation(
                    out=sl[:], in_=e[:], func=Act.Ln, bias=1.0,
                    accum_out=stats_act[:, c:c + 1])

            # ---- gather the GT cells: (128, 16, 18)
            cells = mp.tile([P, JJ * 18], f32)
            cells3 = cells[:].rearrange("p (j c) -> p j c", c=18)
            for j in range(JJ):
                nc.gpsimd.indirect_dma_start(
                    out=cells[:, j * 18:(j + 1) * 18],
                    out_offset=None,
                    in_=pred[:],
                    in_offset=bass.IndirectOffsetOnAxis(
                        ap=rowi[:, j:j + 1], axis=0),
                )

            # ---- box decode
            # sigmoid(tx,ty) via exp(-x) -> 1/(1+e)
            txy_in = cells3[:, :, 1:11].rearrange(
                "p j (k f) -> p j k f", k=2)[:, :, :, 0:2]
            exy = mp.tile([P, 4 * JJ], f32)
            exy4 = exy[:].rearrange("p (j k c) -> p j k c", k=2, c=2)
            nc.scalar.activation(out=exy4, in_=txy_in, func=Act.Exp, scale=-1.0)
            den = mp.tile([P, 4 * JJ], f32)
            nc.vector.tensor_scalar(den[:], exy[:], 1.0, None, Op.add)
            sgm = mp.tile([P, 4 * JJ], f32)
            nc.vector.reciprocal(sgm[:], den[:])
            sgm4 = sgm[:].rearrange("p (j k c) -> p j k c", k=2, c=2)

            # px = (sigmoid + gj) * fl(1/52), matching the reference's order
            gjib = gji[:].rearrange(
                "p (j c) -> p j c", c=2).unsqueeze(2).to_broadcast([P, JJ, 2, 2])
            sgp = mp.tile([P, 4 * JJ], f32)
            sgp4 = sgp[:].rearrange("p (j k c) -> p j k c", k=2, c=2)
            nc.vector.tensor_tensor(sgp4, sgm4, gjib, op=Op.add)
            pxy = mp.tile([P, 4 * JJ], f32)
            pxy4 = pxy[:].rearrange("p (j k c) -> p j k c", k=2, c=2)
            nc.vector.tensor_scalar(pxy[:], sgp[:], 1.0 / S, None, Op.mult)

            twh_in = cells3[:, :, 3:13].rearrange(
                "p j (k f) -> p j k f", k=2)[:, :, :, 0:2]
            ewh = mp.tile([P, 4 * JJ], f32)
            ewh4 = ewh[:].rearrange("p (j k c) -> p j k c", k=2, c=2)
            nc.scalar.activation(out=ewh4, in_=twh_in, func=Act.Exp)
            pwh = mp.tile([P, 4 * JJ], f32)
            nc.vector.tensor_scalar(pwh[:], ewh[:], 1.0, None, Op.min)
            pwh4 = pwh[:].rearrange("p (j k c) -> p j k c", k=2, c=2)
            pwh2 = mp.tile([P, 4 * JJ], f32)
            nc.vector.tensor_scalar(pwh2[:], pwh[:], 0.5, None, Op.mult)

            p1 = mp.tile([P, 4 * JJ], f32)
            nc.vector.tensor_tensor(p1[:], pxy[:], pwh2[:], op=Op.subtract)
            p14 = p1[:].rearrange("p (j k c) -> p j k c", k=2, c=2)
            p2 = mp.tile([P, 4 * JJ], f32)
            nc.vector.tensor_tensor(p2[:], pxy[:], pwh2[:], op=Op.add)
            p24 = p2[:].rearrange("p (j k c) -> p j k c", k=2, c=2)

            wh2g = mp.tile([P, 2 * JJ], f32)
            wh2g3 = wh2g[:].rearrange("p (j c) -> p j c", c=2)
            nc.vector.tensor_scalar(wh2g3, gtb3[:, :, 2:4], 0.5, None, Op.mult)
            g1 = mp.tile([P, 2 * JJ], f32)
            g13 = g1[:].rearrange("p (j c) -> p j c", c=2)
            nc.vector.tensor_tensor(g13, gtb3[:, :, 0:2], wh2g3, op=Op.subtract)
            g2 = mp.tile([P, 2 * JJ], f32)
            g23 = g2[:].rearrange("p (j c) -> p j c", c=2)
            nc.vector.tensor_tensor(g23, gtb3[:, :, 0:2], wh2g3, op=Op.add)
            g1b = g13.unsqueeze(2).to_broadcast([P, JJ, 2, 2])
            g2b = g23.unsqueeze(2).to_broadcast([P, JJ, 2, 2])

            lo = mp.tile([P, 4 * JJ], f32)
            lo4 = lo[:].rearrange("p (j k c) -> p j k c", k=2, c=2)
            nc.vector.tensor_tensor(lo4, p14, g1b, op=Op.max)
            hi = mp.tile([P, 4 * JJ], f32)
            hi4 = hi[:].rearrange("p (j k c) -> p j k c", k=2, c=2)
            nc.vector.tensor_tensor(hi4, p24, g2b, op=Op.min)
            iwr = mp.tile([P, 4 * JJ], f32)
            nc.vector.tensor_tensor(iwr[:], hi[:], lo[:], op=Op.subtract)
            iwh = mp.tile([P, 4 * JJ], f32)
            nc.vector.tensor_scalar(iwh[:], iwr[:], 0.0, None, Op.max)
            iwh4 = iwh[:].rearrange("p (j k c) -> p j k c", k=2, c=2)

            inter = mp.tile([P, 2 * JJ], f32)
            inter3 = inter[:].rearrange("p (j k) -> p j k", k=2)
            nc.vector.tensor_tensor(
                inter3, iwh4[:, :, :, 0], iwh4[:, :, :, 1], op=Op.mult)
            # areas and union with the reference's exact f32 operation order:
            # a1 = (x2-x1)*(y2-y1), a2 from gt corners,
            # union = ((a1 + a2) - inter) + EPS, iou = inter/union
            ad = mp.tile([P, 4 * JJ], f32)
            nc.vector.tensor_tensor(ad[:], p2[:], p1[:], op=Op.subtract)
            ad4 = ad[:].rearrange("p (j k c) -> p j k c", k=2, c=2)
            a1 = mp.tile([P, 2 * JJ], f32)
            a13 = a1[:].rearrange("p (j k) -> p j k", k=2)
            nc.vector.tensor_tensor(
                a13, ad4[:, :, :, 0], ad4[:, :, :, 1], op=Op.mult)
            gd = mp.tile([P, 2 * JJ], f32)
            gd3 = gd[:].rearrange("p (j c) -> p j c", c=2)
            nc.vector.tensor_tensor(gd3, g23, g13, op=Op.subtract)
            a2 = mp.tile([P, JJ], f32)
            nc.vector.tensor_tensor(
                a2[:], gd3[:, :, 0], gd3[:, :, 1], op=Op.mult)
            a2b = a2[:].unsqueeze(2).to_broadcast([P, JJ, 2])
            u1 = mp.tile([P, 2 * JJ], f32)
            u13 = u1[:].rearrange("p (j k) -> p j k", k=2)
            nc.vector.tensor_tensor(u13, a13, a2b, op=Op.add)
            u2 = mp.tile([P, 2 * JJ], f32)
            nc.vector.tensor_tensor(u2[:], u1[:], inter[:], op=Op.subtract)
            union = mp.tile([P, 2 * JJ], f32)
            nc.vector.tensor_scalar(union[:], u2[:], EPS, None, Op.add)
            rcpu = mp.tile([P, 2 * JJ], f32)
            nc.vector.reciprocal(rcpu[:], union[:])
            iou = mp.tile([P, 2 * JJ], f32)
            nc.vector.tensor_tensor(iou[:], inter[:], rcpu[:], op=Op.mult)
            iou3 = iou[:].rearrange("p (j k) -> p j k", k=2)
            sel = mp.tile([P, JJ], f32)
            nc.vector.tensor_tensor(
                sel[:], iou3[:, :, 1], iou3[:, :, 0], op=Op.is_gt)
            selb = sel[:].unsqueeze(2).to_broadcast([P, JJ, 2])

            def lerp_pick(v1, v0, mask_ap, width):
                d = mp.tile([P, width * JJ], f32)
                dv = d[:].rearrange("p (j c) -> p j c", c=width) if width > 1 else d[:]
                nc.vector.tensor_tensor(dv, v1, v0, op=Op.subtract)
                m = mp.tile([P, width * JJ], f32)
                mv = m[:].rearrange("p (j c) -> p j c", c=width) if width > 1 else m[:]
                nc.vector.tensor_tensor(mv, dv, mask_ap, op=Op.mult)
                b = mp.tile([P, width * JJ], f32)
                bv = b[:].rearrange("p (j c) -> p j c", c=width) if width > 1 else b[:]
                nc.vector.tensor_tensor(bv, mv, v0, op=Op.add)
                return b, bv

            bxy, bxy3 = lerp_pick(pxy4[:, :, 1, :], pxy4[:, :, 0, :], selb, 2)
            bwh, bwh3 = lerp_pick(pwh4[:, :, 1, :], pwh4[:, :, 0, :], selb, 2)
            btob, _ = lerp_pick(cells3[:, :, 5], cells3[:, :, 0], sel[:], 1)

            # ---- coord loss
            dxy = mp.tile([P, 2 * JJ], f32)
            dxy3 = dxy[:].rearrange("p (j c) -> p j c", c=2)
            nc.vector.tensor_tensor(dxy3, bxy3, gtb3[:, :, 0:2], op=Op.subtract)
            dxy2 = mp.tile([P, 2 * JJ], f32)
            nc.vector.tensor_tensor(dxy2[:], dxy[:], dxy[:], op=Op.mult)
            cdxy = mp.tile([P, JJ], f32)
            nc.vector.tensor_reduce(
                cdxy[:], dxy2[:].rearrange("p (j c) -> p j c", c=2),
                axis=AxX, op=Op.add)
            # sqrt(x + EPS) = exp(0.5 * ln(x + EPS)); keeps ACT on one table set
            lnp = mp.tile([P, 2 * JJ], f32)
            nc.scalar.activation(out=lnp[:], in_=bwh[:], func=Act.Ln, bias=cst[:, 282:283])
            syp = mp.tile([P, 2 * JJ], f32)
            nc.scalar.activation(out=syp[:], in_=lnp[:], func=Act.Exp, scale=0.5)
            lng = mp.tile([P, 2 * JJ], f32)
            lng3 = lng[:].rearrange("p (j c) -> p j c", c=2)
            nc.scalar.activation(
                out=lng3, in_=gtb3[:, :, 2:4], func=Act.Ln, bias=cst[:, 282:283])
            syg = mp.tile([P, 2 * JJ], f32)
            nc.scalar.activation(out=syg[:], in_=lng[:], func=Act.Exp, scale=0.5)
            dwh = mp.tile([P, 2 * JJ], f32)
            nc.vector.tensor_tensor(dwh[:], syp[:], syg[:], op=Op.subtract)
            dwh2 = mp.tile([P, 2 * JJ], f32)
            nc.vector.tensor_tensor(dwh2[:], dwh[:], dwh[:], op=Op.mult)
            cdwh = mp.tile([P, JJ], f32)
            nc.vector.tensor_reduce(
                cdwh[:], dwh2[:].rearrange("p (j c) -> p j c", c=2),
                axis=AxX, op=Op.add)
            coordt = mp.tile([P, JJ], f32)
            nc.vector.tensor_tensor(coordt[:], cdxy[:], cdwh[:], op=Op.add)
            coordv = mp.tile([P, JJ], f32)
            nc.vector.scalar_tensor_tensor(
                out=coordv[:], in0=coordt[:], scalar=1.0, in1=gtv[:],
                op0=Op.mult, op1=Op.mult, accum_out=stats_dve[:, 0:1])

            # ---- obj loss: softplus(-logit)
            eo = mp.tile([P, JJ], f32)
            nc.scalar.activation(out=eo[:], in_=btob[:], func=Act.Exp, scale=-1.0)
            so = mp.tile([P, JJ], f32)
            nc.scalar.activation(out=so[:], in_=eo[:], func=Act.Ln, bias=1.0)
            objv = mp.tile([P, JJ], f32)
            nc.vector.scalar_tensor_tensor(
                out=objv[:], in0=so[:], scalar=1.0, in1=gtv[:],
                op0=Op.mult, op1=Op.mult, accum_out=stats_dve[:, 1:2])

            # ---- class NLL
            clsl = cells3[:, :, 10:18]
            mx = mp.tile([P, JJ], f32)
            nc.vector.tensor_reduce(mx[:], clsl, axis=AxX, op=Op.max)
            sh = mp.tile([P, NCLS * JJ], f32)
            sh3 = sh[:].rearrange("p (j c) -> p j c", c=NCLS)
            mxb = mx[:].unsqueeze(2).to_broadcast([P, JJ, NCLS])
            nc.vector.tensor_tensor(sh3, clsl, mxb, op=Op.subtract)
            ex = mp.tile([P, NCLS * JJ], f32)
            nc.scalar.activation(out=ex[:], in_=sh[:], func=Act.Exp)
            sm = mp.tile([P, JJ], f32)
            nc.vector.tensor_reduce(
                sm[:], ex[:].rearrange("p (j c) -> p j c", c=NCLS),
                axis=AxX, op=Op.add)
            ls = mp.tile([P, JJ], f32)
            nc.scalar.activation(out=ls[:], in_=sm[:], func=Act.Ln)
            oh = mp.tile([P, NCLS * JJ], f32)
            oh3 = oh[:].rearrange("p (j c) -> p j c", c=NCLS)
            gtlb = gtl[:].unsqueeze(2).to_broadcast([P, JJ, NCLS])
            iotb = cst[:, 16:24].unsqueeze(1).to_broadcast([P, JJ, NCLS])
            nc.vector.tensor_tensor(oh3, gtlb, iotb, op=Op.is_equal)
            pick = mp.tile([P, NCLS * JJ], f32)
            nc.vector.tensor_tensor(pick[:], oh[:], sh[:], op=Op.mult)
            lab = mp.tile([P, JJ], f32)
            nc.vector.tensor_reduce(
                lab[:], pick[:].rearrange("p (j c) -> p j c", c=NCLS),
                axis=AxX, op=Op.add)
            nll = mp.tile([P, JJ], f32)
            nc.vector.tensor_tensor(nll[:], ls[:], lab[:], op=Op.subtract)
            nllv = mp.tile([P, JJ], f32)
            nc.vector.scalar_tensor_tensor(
                out=nllv[:], in0=nll[:], scalar=1.0, in1=gtv[:],
                op0=Op.mult, op1=Op.mult, accum_out=stats_dve[:, 2:3])

            # ---- noobj correction: once per distinct GT cell, softplus of
            #      both objectness logits of the gathered cell
            rowm_a = mp.tile([P, JJ], f32)
            nc.vector.scalar_tensor_tensor(
                out=rowm_a[:], in0=gtv[:], scalar=BIG, in1=rowf[:],
                op0=Op.mult, op1=Op.add)
            rowm = mp.tile([P, JJ], f32)
            nc.vector.tensor_scalar(rowm[:], rowm_a[:], -BIG, None, Op.add)
            rowp = mp.tile([P, JJ], f32)
            nc.vector.stream_shuffle(out=rowp[:], in_=rowm[:], mask=shuffle_mask)

            rmj = rowm[:].unsqueeze(2).to_broadcast([P, JJ, JJ])
            rmq = rowm[:].unsqueeze(1).to_broadcast([P, JJ, JJ])
            rpq = rowp[:].unsqueeze(1).to_broadcast([P, JJ, JJ])
            cmps = mp.tile([P, JJ * JJ], f32)
            cmps3 = cmps[:].rearrange("p (j q) -> p j q", q=JJ)
            nc.vector.tensor_tensor(cmps3, rmj, rmq, op=Op.is_equal)
            prods = mp.tile([P, JJ * JJ], f32)
            nc.vector.tensor_tensor(
                prods[:], cmps[:], cst[:, 24:280], op=Op.mult)
            cnts = mp.tile([P, JJ], f32)
            nc.vector.tensor_reduce(
                cnts[:], prods[:].rearrange("p (j q) -> p j q", q=JJ),
                axis=AxX, op=Op.add)
            cmpp = mp.tile([P, JJ * JJ], f32)
            cmpp3 = cmpp[:].rearrange("p (j q) -> p j q", q=JJ)
            nc.vector.tensor_tensor(cmpp3, rmj, rpq, op=Op.is_equal)
            cntp = mp.tile([P, JJ], f32)
            nc.vector.tensor_reduce(
                cntp[:], cmpp[:].rearrange("p (j q) -> p j q", q=JJ),
                axis=AxX, op=Op.add)
            dup = mp.tile([P, JJ], f32)
            nc.vector.scalar_tensor_tensor(
                out=dup[:], in0=cntp[:], scalar=cst[:, 280:281], in1=cnts[:],
                op0=Op.mult, op1=Op.add)
            wd = mp.tile([P, JJ], f32)
            nc.vector.tensor_scalar(wd[:], dup[:], 0.0, None, Op.is_equal)
            wv = mp.tile([P, JJ], f32)
            nc.vector.tensor_tensor(wv[:], wd[:], gtv[:], op=Op.mult)

            ec = mp.tile([P, 2 * JJ], f32)
            ec3 = ec[:].rearrange("p (j c) -> p j c", c=2)
            nc.scalar.activation(
                out=ec3, in_=cells3[:, :, 0:10:5], func=Act.Exp)
            scn = mp.tile([P, 2 * JJ], f32)
            nc.scalar.activation(out=scn[:], in_=ec[:], func=Act.Ln, bias=1.0)
            spc = mp.tile([P, JJ], f32)
            nc.vector.tensor_reduce(
                spc[:], scn[:].rearrange("p (j c) -> p j c", c=2),
                axis=AxX, op=Op.add)
            corrv = mp.tile([P, JJ], f32)
            nc.vector.scalar_tensor_tensor(
                out=corrv[:], in0=spc[:], scalar=1.0, in1=wv[:],
                op0=Op.mult, op1=Op.mult, accum_out=stats_dve[:, 3:4])

            if debug:
                dbg = mp.tile([P, 12 * JJ], f32)
                nc.vector.tensor_copy(out=dbg[:, 0:16], in_=sel[:])
                nc.vector.tensor_copy(out=dbg[:, 16:48], in_=bxy[:])
                nc.vector.tensor_copy(out=dbg[:, 48:80], in_=bwh[:])
                nc.vector.tensor_copy(out=dbg[:, 80:96], in_=btob[:])
                nc.vector.tensor_copy(out=dbg[:, 96:112], in_=coordt[:])
                nc.vector.tensor_copy(out=dbg[:, 112:128], in_=so[:])
                nc.vector.tensor_copy(out=dbg[:, 128:144], in_=rowf[:])
                nc.vector.tensor_copy(out=dbg[:, 144:176], in_=gji[:])
                nc.vector.tensor_copy(out=dbg[:, 176:192], in_=nll[:])
                nc.sync.dma_start(out=dbg_d[:], in_=dbg[:])

            # ---- cross-partition reduce: ones^T @ stats
            ps = pp.tile([1, 4 + NCHUNK], f32)
            nc.tensor.matmul(
                out=ps[:, 0:4], lhsT=cst[:, 281:282], rhs=stats_dve[:],
                start=True, stop=True)
            nc.tensor.matmul(
                out=ps[:, 4:4 + NCHUNK], lhsT=cst[:, 281:282], rhs=stats_act[:],
                start=True, stop=True)
            outt = mp.tile([1, 4 + NCHUNK], f32)
            nc.vector.tensor_copy(out=outt[:], in_=ps[:])
            nc.sync.dma_start(out=out_d[:], in_=outt[:])

    nc.compile()
    return nc


_CONSTS = _build_consts()
_NC_CACHE = {}


def _get_program(for_sim: bool = False) -> bass.Bass:
    key = bool(for_sim)
    if key not in _NC_CACHE:
        _NC_CACHE[key] = build_program(for_sim)
    return _NC_CACHE[key]


def make_in_maps(predictions, gt_boxes, gt_labels, gt_valid):
    predictions = np.ascontiguousarray(np.asarray(predictions), np.float32)
    gtb = np.ascontiguousarray(np.asarray(gt_boxes), np.float32)
    gtl = np.asarray(gt_labels).astype(np.float32)
    gtv = np.asarray(gt_valid).astype(np.float32)
    in_maps = []
    for c in range(NCORES):
        sl = slice(c * NIMG, (c + 1) * NIMG)
        in_maps.append({
            "pred": predictions[sl].reshape(ROWS, 18),
            "gtb": gtb[sl].reshape(NG, 4).reshape(P, JJ * 4),
            "gtl": gtl[sl].reshape(NG).reshape(P, JJ),
            "gtv": gtv[sl].reshape(NG).reshape(P, JJ),
            "consts": _CONSTS,
        })
    return in_maps


def combine_outputs(outs):
    """outs: list of (1, 12) per-core partials -> 5-tuple of f32 scalars."""
    t = np.stack([np.asarray(o).reshape(4 + NCHUNK) for o in outs]).astype(np.float64)
    s = t.sum(0)
    coord, obj, cls, corr = s[0], s[1], s[2], s[3]
    noobj = s[4:4 + NCHUNK].sum() - corr
    total = (LAMBDA_COORD * coord + obj + LAMBDA_NOOBJ * noobj + cls) / BATCH
    return (np.float32(total), np.float32(coord / BATCH),
            np.float32(obj / BATCH), np.float32(noobj / BATCH),
            np.float32(cls / BATCH))


def kernel(predictions, gt_boxes, gt_labels, gt_valid):
    from concourse.bass_utils import run_bass_kernel_spmd

    nc = _get_program(for_sim=False)
    in_maps = make_in_maps(predictions, gt_boxes, gt_labels, gt_valid)
    res = run_bass_kernel_spmd(nc, in_maps, list(range(NCORES))).results
    return combine_outputs([r["out"] for r in res])


# revision 25
# speedup vs baseline: 1.1046x; 1.1046x over previous
"""Trainium2 Bass kernel for YOLO-style DetectionLoss.

Contract: kernel(**inputs) takes the FULL inputs (batch 512) and returns the
full output (5-tuple of f32 scalars), sharding batch-wise across 8 NeuronCores.

Per-core device program (64 images, 2048 GTs):
  - stream the predictions shard (12.5 MB) through SBUF in 8 contiguous DMAs,
    accumulating sum(softplus(objectness)) over channels {0,5} of every cell
    (ACT exp -> ln(1+x) with accum_out)
  - compute GT cell row indices on-device, gather the 2048 GT cells with one
    indirect DMA
  - decode boxes (sigmoid via exp+reciprocal; min(exp,1)), pick the
    responsible box via cross-multiplied IoU comparison (no divide), and
    accumulate coord / obj / class losses
  - deduplicate cells holding >=1 GT (pairwise compare within each image via
    a partner-partition stream_shuffle) and subtract their softplus terms
    from the noobj sum
  - reduce all partials across partitions with a ones-vector matmul -> 12
    scalars, summed on host across the 8 cores.
"""
import sys

sys.path.insert(0, "/opt/trn_rl_repo")

import numpy as np

import concourse.bass as bass
import concourse.tile as tile
from concourse import bacc, mybir

S = 52
NBOX = 2
NCLS = 8
EPS = 1e-6
LAMBDA_COORD = 5.0
LAMBDA_NOOBJ = 0.5
BATCH = 512
N_GT = 32
NCORES = 8
NIMG = BATCH // NCORES          # 64 images per core
CELLS = S * S                   # 2704
ROWS = NIMG * CELLS             # 173056 rows of 18 floats per core
NG = NIMG * N_GT                # 2048 GTs per core
P = 128
JJ = NG // P                    # 16 GTs per partition
NCHUNK = 8                      # streaming chunks
FREE = NIMG * CELLS * 18 // NCHUNK // P   # 3042 f32 per partition per chunk
BIG = 1.0e7                     # invalid-GT row sentinel offset (exact in f32)

f32 = mybir.dt.float32
i32 = mybir.dt.int32
Act = mybir.ActivationFunctionType
Op = mybir.AluOpType
AxX = mybir.AxisListType.X

# consts layout (128, 288):
#   [0:16)    row_base   = (g // 32) * 2704, g = p*16+j
#   [16:24)   iota over classes 0..7
#   [24:280)  lower-strict-triangular mask tri[j*16+q] = 1.0 if q < j
#   [280]     parity = p % 2
#   [281]     ones
#   [282]     EPS (1e-6)
CONST_W = 288


def _build_consts() -> np.ndarray:
    c = np.zeros((P, CONST_W), np.float32)
    p = np.arange(P)[:, None]
    j = np.arange(JJ)[None, :]
    g = p * JJ + j
    c[:, 0:16] = (g // N_GT) * CELLS
    c[:, 16:24] = np.arange(NCLS)[None, :]
    tri = (np.arange(JJ)[None, :] < np.arange(JJ)[:, None]).astype(np.float32)
    c[:, 24:280] = tri.reshape(-1)[None, :]
    c[:, 280] = (np.arange(P) % 2).astype(np.float32)
    c[:, 281] = 1.0
    c[:, 282] = EPS
    return c


_ACT_PATCHED = False


def _force_single_act_table():
    """Make the act-table-load pass place every activation in
    natural_log_exp_and_others (covers Exp+Ln), so the kernel pays one
    ACT_TABLE_LOAD instead of thrashing between per-function sets.
    Indices of the table dict are preserved (walrus resolves the id
    against the same act_info.json)."""
    global _ACT_PATCHED
    if _ACT_PATCHED:
        return
    from concourse import hw_specs

    orig = hw_specs.get_activation_tables

    def patched(arch):
        t = orig(arch)
        keep = "natural_log_exp_and_others"
        if keep not in t:
            return t
        return {k: (v if k == keep else set()) for k, v in t.items()}

    hw_specs.get_activation_tables = patched
    bacc.get_activation_tables = patched
    _ACT_PATCHED = True


def build_program(for_sim: bool = False, debug: bool = False) -> bass.Bass:
    _force_single_act_table()
    nc = bacc.Bacc(None, target_bir_lowering=False)

    pred = nc.dram_tensor("pred", [ROWS, 18], f32, kind="ExternalInput")
    gtb_d = nc.dram_tensor("gtb", [P, JJ * 4], f32, kind="ExternalInput")
    gtl_d = nc.dram_tensor("gtl", [P, JJ], f32, kind="ExternalInput")
    gtv_d = nc.dram_tensor("gtv", [P, JJ], f32, kind="ExternalInput")
    cst_d = nc.dram_tensor("consts", [P, CONST_W], f32, kind="ExternalInput")
    out_d = nc.dram_tensor("out", [1, 4 + NCHUNK], f32, kind="ExternalOutput")
    if debug:
        dbg_d = nc.dram_tensor("dbg", [P, 12 * JJ], f32, kind="ExternalOutput")

    shuffle_mask = []
    for i in range(0, 32, 2):
        shuffle_mask += [i + 1, i]

    with tile.TileContext(nc) as tc:
        with (
            tc.tile_pool(name="main", bufs=1) as mp,
            tc.tile_pool(name="stream", bufs=8) as sp,
            tc.tile_pool(name="psum", bufs=1, space="PSUM") as pp,
        ):
            # ---- small input DMAs
            cst = mp.tile([P, CONST_W], f32)
            nc.sync.dma_start(out=cst[:], in_=cst_d[:])
            gtb = mp.tile([P, JJ * 4], f32)
            nc.sync.dma_start(out=gtb[:], in_=gtb_d[:])
            gtl = mp.tile([P, JJ], f32)
            nc.sync.dma_start(out=gtl[:], in_=gtl_d[:])
            gtv = mp.tile([P, JJ], f32)
            nc.sync.dma_start(out=gtv[:], in_=gtv_d[:])

            gtb3 = gtb[:].rearrange("p (j c) -> p j c", c=4)
            stats_dve = mp.tile([P, 4], f32)   # coord, obj, cls, corr
            stats_act = mp.tile([P, NCHUNK], f32)

            # ---- GT cell indices: gj from cx, gi from cy (layout (j, c):
            #      c=0 -> gj, c=1 -> gi)
            t52 = mp.tile([P, 2 * JJ], f32)
            t52v = t52[:].rearrange("p (j c) -> p j c", c=2)
            nc.vector.tensor_scalar(t52v, gtb3[:, :, 0:2], 52.0, None, Op.mult)
            # floor(x): int cast rounds-to-nearest on HW (truncates in sim);
            # r - (r > x) is exact floor under either behavior for x >= 0
            gjii = mp.tile([P, 2 * JJ], i32)
            nc.vector.tensor_copy(out=gjii[:], in_=t52[:])
            gjif = mp.tile([P, 2 * JJ], f32)
            nc.vector.tensor_copy(out=gjif[:], in_=gjii[:])
            gadj = mp.tile([P, 2 * JJ], f32)
            nc.vector.tensor_tensor(gadj[:], gjif[:], t52[:], op=Op.is_gt)
            gjfl = mp.tile([P, 2 * JJ], f32)
            nc.vector.tensor_tensor(gjfl[:], gjif[:], gadj[:], op=Op.subtract)
            gji = mp.tile([P, 2 * JJ], f32)
            nc.vector.tensor_scalar(gji[:], gjfl[:], 51.0, 0.0, Op.min, Op.max)
            gji3 = gji[:].rearrange("p (j c) -> p j c", c=2)

            # row = row_base + gi*52 + gj
            rowa = mp.tile([P, JJ], f32)
            nc.vector.scalar_tensor_tensor(
                out=rowa[:], in0=gji3[:, :, 1], scalar=52.0, in1=gji3[:, :, 0],
                op0=Op.mult, op1=Op.add)
            rowf = mp.tile([P, JJ], f32)
            nc.vector.tensor_tensor(
                out=rowf[:], in0=rowa[:], in1=cst[:, 0:16], op=Op.add)
            rowi = mp.tile([P, JJ], i32)
            nc.vector.tensor_copy(out=rowi[:], in_=rowf[:])

            # ---- streaming softplus over all objectness logits
            predv = pred[:].rearrange("(c p f) d -> c p (f d)", c=NCHUNK, p=P)
            for c in range(NCHUNK):
                st = sp.tile([P, FREE], f32, tag="st")
                nc.sync.dma_start(out=st[:], in_=predv[c])
                st3 = st[:].rearrange("p (f d) -> p f d", d=18)
                e = sp.tile([P, FREE // 9], f32, tag="spe")
                e3 = e[:].rearrange("p (f d) -> p f d", d=2)
                nc.scalar.activation(out=e3, in_=st3[:, :, 0:10:5], func=Act.Exp)
                sl = sp.tile([P, FREE // 9], f32, tag="spl")
                nc.scalar.activation(
                    out=sl[:], in_=e[:], func=Act.Ln, bias=1.0,
                    accum_out=stats_act[:, c:c + 1])

            # ---- gather the GT cells: (128, 16, 18)
            cells = mp.tile([P, JJ * 18], f32)
            cells3 = cells[:].rearrange("p (j c) -> p j c", c=18)
            for j in range(JJ):
                nc.gpsimd.indirect_dma_start(
                    out=cells[:, j * 18:(j + 1) * 18],
                    out_offset=None,
                    in_=pred[:],
                    in_offset=bass.IndirectOffsetOnAxis(
                        ap=rowi[:, j:j + 1], axis=0),
                )

            # ---- box decode
            # sigmoid(tx,ty) via exp(-x) -> 1/(1+e)
            txy_in = cells3[:, :, 1:11].rearrange(
                "p j (k f) -> p j k f", k=2)[:, :, :, 0:2]
            exy = mp.tile([P, 4 * JJ], f32)
            exy4 = exy[:].rearrange("p (j k c) -> p j k c", k=2, c=2)
            nc.scalar.activation(out=exy4, in_=txy_in, func=Act.Exp, scale=-1.0)
            den = mp.tile([P, 4 * JJ], f32)
            nc.vector.tensor_scalar(den[:], exy[:], 1.0, None, Op.add)
            sgm = mp.tile([P, 4 * JJ], f32)
            nc.vector.reciprocal(sgm[:], den[:])
            sgm4 = sgm[:].rearrange("p (j k c) -> p j k c", k=2, c=2)

            # px = (sigmoid + gj) * fl(1/52), matching the reference's order
            gjib = gji[:].rearrange(
                "p (j c) -> p j c", c=2).unsqueeze(2).to_broadcast([P, JJ, 2, 2])
            sgp = mp.tile([P, 4 * JJ], f32)
            sgp4 = sgp[:].rearrange("p (j k c) -> p j k c", k=2, c=2)
            nc.vector.tensor_tensor(sgp4, sgm4, gjib, op=Op.add)
            pxy = mp.tile([P, 4 * JJ], f32)
            pxy4 = pxy[:].rearrange("p (j k c) -> p j k c", k=2, c=2)
            nc.vector.tensor_scalar(pxy[:], sgp[:], 1.0 / S, None, Op.mult)

            twh_in = cells3[:, :, 3:13].rearrange(
                "p j (k f) -> p j k f", k=2)[:, :, :, 0:2]
            ewh = mp.tile([P, 4 * JJ], f32)
            ewh4 = ewh[:].rearrange("p (j k c) -> p j k c", k=2, c=2)
            nc.scalar.activation(out=ewh4, in_=twh_in, func=Act.Exp)
            pwh = mp.tile([P, 4 * JJ], f32)
            nc.vector.tensor_scalar(pwh[:], ewh[:], 1.0, None, Op.min)
            pwh4 = pwh[:].rearrange("p (j k c) -> p j k c", k=2, c=2)
            pwh2 = mp.tile([P, 4 * JJ], f32)
            nc.vector.tensor_scalar(pwh2[:], pwh[:], 0.5, None, Op.mult)

            p1 = mp.tile([P, 4 * JJ], f32)
            nc.vector.tensor_tensor(p1[:], pxy[:], pwh2[:], op=Op.subtract)
            p14 = p1[:].rearrange("p (j k c) -> p j k c", k=2, c=2)
            p2 = mp.tile([P, 4 * JJ], f32)
            nc.vector.tensor_tensor(p2[:], pxy[:], pwh2[:], op=Op.add)
            p24 = p2[:].rearrange("p (j k c) -> p j k c", k=2, c=2)

            wh2g = mp.tile([P, 2 * JJ], f32)
            wh2g3 = wh2g[:].rearrange("p (j c) -> p j c", c=2)
            nc.vector.tensor_scalar(wh2g3, gtb3[:, :, 2:4], 0.5, None, Op.mult)
            g1 = mp.tile([P, 2 * JJ], f32)
            g13 = g1[:].rearrange("p (j c) -> p j c", c=2)
            nc.vector.tensor_tensor(g13, gtb3[:, :, 0:2], wh2g3, op=Op.subtract)
            g2 = mp.tile([P, 2 * JJ], f32)
            g23 = g2[:].rearrange("p (j c) -> p j c", c=2)
            nc.vector.tensor_tensor(g23, gtb3[:, :, 0:2], wh2g3, op=Op.add)
            g1b = g13.unsqueeze(2).to_broadcast([P, JJ, 2, 2])
            g2b = g23.unsqueeze(2).to_broadcast([P, JJ, 2, 2])

            lo = mp.tile([P, 4 * JJ], f32)
            lo4 = lo[:].rearrange("p (j k c) -> p j k c", k=2, c=2)
            nc.vector.tensor_tensor(lo4, p14, g1b, op=Op.max)
            hi = mp.tile([P, 4 * JJ], f32)
            hi4 = hi[:].rearrange("p (j k c) -> p j k c", k=2, c=2)
            nc.vector.tensor_tensor(hi4, p24, g2b, op=Op.min)
            iwr = mp.tile([P, 4 * JJ], f32)
            nc.vector.tensor_tensor(iwr[:], hi[:], lo[:], op=Op.subtract)
            iwh = mp.tile([P, 4 * JJ], f32)
            nc.vector.tensor_scalar(iwh[:], iwr[:], 0.0, None, Op.max)
            iwh4 = iwh[:].rearrange("p (j k c) -> p j k c", k=2, c=2)

            inter = mp.tile([P, 2 * JJ], f32)
            inter3 = inter[:].rearrange("p (j k) -> p j k", k=2)
            nc.vector.tensor_tensor(
                inter3, iwh4[:, :, :, 0], iwh4[:, :, :, 1], op=Op.mult)
            # areas and union with the reference's exact f32 operation order:
            # a1 = (x2-x1)*(y2-y1), a2 from gt corners,
            # union = ((a1 + a2) - inter) + EPS, iou = inter/union
            ad = mp.tile([P, 4 * JJ], f32)
            nc.vector.tensor_tensor(ad[:], p2[:], p1[:], op=Op.subtract)
            ad4 = ad[:].rearrange("p (j k c) -> p j k c", k=2, c=2)
            a1 = mp.tile([P, 2 * JJ], f32)
            a13 = a1[:].rearrange("p (j k) -> p j k", k=2)
            nc.vector.tensor_tensor(
                a13, ad4[:, :, :, 0], ad4[:, :, :, 1], op=Op.mult)
            gd = mp.tile([P, 2 * JJ], f32)
            gd3 = gd[:].rearrange("p (j c) -> p j c", c=2)
            nc.vector.tensor_tensor(gd3, g23, g13, op=Op.subtract)
            a2 = mp.tile([P, JJ], f32)
            nc.vector.tensor_tensor(
                a2[:], gd3[:, :, 0], gd3[:, :, 1], op=Op.mult)
            a2b = a2[:].unsqueeze(2).to_broadcast([P, JJ, 2])
            u1 = mp.tile([P, 2 * JJ], f32)
            u13 = u1[:].rearrange("p (j k) -> p j k", k=2)
            nc.vector.tensor_tensor(u13, a13, a2b, op=Op.add)
            u2 = mp.tile([P, 2 * JJ], f32)
            nc.vector.tensor_tensor(u2[:], u1[:], inter[:], op=Op.subtract)
            union = mp.tile([P, 2 * JJ], f32)
            nc.vector.tensor_scalar(union[:], u2[:], EPS, None, Op.add)
            rcpu = mp.tile([P, 2 * JJ], f32)
            nc.vector.reciprocal(rcpu[:], union[:])
            iou = mp.tile([P, 2 * JJ], f32)
            nc.vector.tensor_tensor(iou[:], inter[:], rcpu[:], op=Op.mult)
            iou3 = iou[:].rearrange("p (j k) -> p j k", k=2)
            sel = mp.tile([P, JJ], f32)
            nc.vector.tensor_tensor(
                sel[:], iou3[:, :, 1], iou3[:, :, 0], op=Op.is_gt)
            selb = sel[:].unsqueeze(2).to_broadcast([P, JJ, 2])

            def lerp_pick(v1, v0, mask_ap, width):
                d = mp.tile([P, width * JJ], f32)
                dv = d[:].rearrange("p (j c) -> p j c", c=width) if width > 1 else d[:]
                nc.vector.tensor_tensor(dv, v1, v0, op=Op.subtract)
                m = mp.tile([P, width * JJ], f32)
                mv = m[:].rearrange("p (j c) -> p j c", c=width) if width > 1 else m[:]
                nc.vector.tensor_tensor(mv, dv, mask_ap, op=Op.mult)
                b = mp.tile([P, width * JJ], f32)
                bv = b[:].rearrange("p (j c) -> p j c", c=width) if width > 1 else b[:]
                nc.vector.tensor_tensor(bv, mv, v0, op=Op.add)
                return b, bv

            bxy, bxy3 = lerp_pick(pxy4[:, :, 1, :], pxy4[:, :, 0, :], selb, 2)
            bwh, bwh3 = lerp_pick(pwh4[:, :, 1, :], pwh4[:, :, 0, :], selb, 2)
            btob, _ = lerp_pick(cells3[:, :, 5], cells3[:, :, 0], sel[:], 1)

            # ---- coord loss
            dxy = mp.tile([P, 2 * JJ], f32)
            dxy3 = dxy[:].rearrange("p (j c) -> p j c", c=2)
            nc.vector.tensor_tensor(dxy3, bxy3, gtb3[:, :, 0:2], op=Op.subtract)
            dxy2 = mp.tile([P, 2 * JJ], f32)
            nc.vector.tensor_tensor(dxy2[:], dxy[:], dxy[:], op=Op.mult)
            cdxy = mp.tile([P, JJ], f32)
            nc.vector.tensor_reduce(
                cdxy[:], dxy2[:].rearrange("p (j c) -> p j c", c=2),
                axis=AxX, op=Op.add)
            # sqrt(x + EPS) = exp(0.5 * ln(x + EPS)); keeps ACT on one table set
            lnp = mp.tile([P, 2 * JJ], f32)
            nc.scalar.activation(out=lnp[:], in_=bwh[:], func=Act.Ln, bias=cst[:, 282:283])
            syp = mp.tile([P, 2 * JJ], f32)
            nc.scalar.activation(out=syp[:], in_=lnp[:], func=Act.Exp, scale=0.5)
            lng = mp.tile([P, 2 * JJ], f32)
            lng3 = lng[:].rearrange("p (j c) -> p j c", c=2)
            nc.scalar.activation(
                out=lng3, in_=gtb3[:, :, 2:4], func=Act.Ln, bias=cst[:, 282:283])
            syg = mp.tile([P, 2 * JJ], f32)
            nc.scalar.activation(out=syg[:], in_=lng[:], func=Act.Exp, scale=0.5)
            dwh = mp.tile([P, 2 * JJ], f32)
            nc.vector.tensor_tensor(dwh[:], syp[:], syg[:], op=Op.subtract)
            dwh2 = mp.tile([P, 2 * JJ], f32)
            nc.vector.tensor_tensor(dwh2[:], dwh[:], dwh[:], op=Op.mult)
            cdwh = mp.tile([P, JJ], f32)
            nc.vector.tensor_reduce(
                cdwh[:], dwh2[:].rearrange("p (j c) -> p j c", c=2),
                axis=AxX, op=Op.add)
            coordt = mp.tile([P, JJ], f32)
            nc.vector.tensor_tensor(coordt[:], cdxy[:], cdwh[:], op=Op.add)
            coordv = mp.tile([P, JJ], f32)
            nc.vector.scalar_tensor_tensor(
                out=coordv[:], in0=coordt[:], scalar=1.0, in1=gtv[:],
                op0=Op.mult, op1=Op.mult, accum_out=stats_dve[:, 0:1])

            # ---- obj loss: softplus(-logit)
            eo = mp.tile([P, JJ], f32)
            nc.scalar.activation(out=eo[:], in_=btob[:], func=Act.Exp, scale=-1.0)
            so = mp.tile([P, JJ], f32)
            nc.scalar.activation(out=so[:], in_=eo[:], func=Act.Ln, bias=1.0)
            objv = mp.tile([P, JJ], f32)
            nc.vector.scalar_tensor_tensor(
                out=objv[:], in0=so[:], scalar=1.0, in1=gtv[:],
                op0=Op.mult, op1=Op.mult, accum_out=stats_dve[:, 1:2])

            # ---- class NLL
            clsl = cells3[:, :, 10:18]
            mx = mp.tile([P, JJ], f32)
            nc.vector.tensor_reduce(mx[:], clsl, axis=AxX, op=Op.max)
            sh = mp.tile([P, NCLS * JJ], f32)
            sh3 = sh[:].rearrange("p (j c) -> p j c", c=NCLS)
            mxb = mx[:].unsqueeze(2).to_broadcast([P, JJ, NCLS])
            nc.vector.tensor_tensor(sh3, clsl, mxb, op=Op.subtract)
            ex = mp.tile([P, NCLS * JJ], f32)
            nc.scalar.activation(out=ex[:], in_=sh[:], func=Act.Exp)
            sm = mp.tile([P, JJ], f32)
            nc.vector.tensor_reduce(
                sm[:], ex[:].rearrange("p (j c) -> p j c", c=NCLS),
                axis=AxX, op=Op.add)
            ls = mp.tile([P, JJ], f32)
            nc.scalar.activation(out=ls[:], in_=sm[:], func=Act.Ln)
            oh = mp.tile([P, NCLS * JJ], f32)
            oh3 = oh[:].rearrange("p (j c) -> p j c", c=NCLS)
            gtlb = gtl[:].unsqueeze(2).to_broadcast([P, JJ, NCLS])
            iotb = cst[:, 16:24].unsqueeze(1).to_broadcast([P, JJ, NCLS])
            nc.vector.tensor_tensor(oh3, gtlb, iotb, op=Op.is_equal)
            pick = mp.tile([P, NCLS * JJ], f32)
            nc.vector.tensor_tensor(pick[:], oh[:], sh[:], op=Op.mult)
            lab = mp.tile([P, JJ], f32)
            nc.vector.tensor_reduce(
                lab[:], pick[:].rearrange("p (j c) -> p j c", c=NCLS),
                axis=AxX, op=Op.add)
            nll = mp.tile([P, JJ], f32)
            nc.vector.tensor_tensor(nll[:], ls[:], lab[:], op=Op.subtract)
            nllv = mp.tile([P, JJ], f32)
            nc.vector.scalar_tensor_tensor(
                out=nllv[:], in0=nll[:], scalar=1.0, in1=gtv[:],
                op0=Op.mult, op1=Op.mult, accum_out=stats_dve[:, 2:3])

            # ---- noobj correction: once per distinct GT cell, softplus of
            #      both objectness logits of the gathered cell
            rowm_a = mp.tile([P, JJ], f32)
            nc.vector.scalar_tensor_tensor(
                out=rowm_a[:], in0=gtv[:], scalar=BIG, in1=rowf[:],
                op0=Op.mult, op1=Op.add)
            rowm = mp.tile([P, JJ], f32)
            nc.vector.tensor_scalar(rowm[:], rowm_a[:], -BIG, None, Op.add)
            rowp = mp.tile([P, JJ], f32)
            nc.vector.stream_shuffle(out=rowp[:], in_=rowm[:], mask=shuffle_mask)

            rmj = rowm[:].unsqueeze(2).to_broadcast([P, JJ, JJ])
            rmq = rowm[:].unsqueeze(1).to_broadcast([P, JJ, JJ])
            rpq = rowp[:].unsqueeze(1).to_broadcast([P, JJ, JJ])
            cmps = mp.tile([P, JJ * JJ], f32)
            cmps3 = cmps[:].rearrange("p (j q) -> p j q", q=JJ)
            nc.vector.tensor_tensor(cmps3, rmj, rmq, op=Op.is_equal)
            prods = mp.tile([P, JJ * JJ], f32)
            nc.vector.tensor_tensor(
                prods[:], cmps[:], cst[:, 24:280], op=Op.mult)
            cnts = mp.tile([P, JJ], f32)
            nc.vector.tensor_reduce(
                cnts[:], prods[:].rearrange("p (j q) -> p j q", q=JJ),
                axis=AxX, op=Op.add)
            cmpp = mp.tile([P, JJ * JJ], f32)
            cmpp3 = cmpp[:].rearrange("p (j q) -> p j q", q=JJ)
            nc.vector.tensor_tensor(cmpp3, rmj, rpq, op=Op.is_equal)
            cntp = mp.tile([P, JJ], f32)
            nc.vector.tensor_reduce(
                cntp[:], cmpp[:].rearrange("p (j q) -> p j q", q=JJ),
                axis=AxX, op=Op.add)
            dup = mp.tile([P, JJ], f32)
            nc.vector.scalar_tensor_tensor(
                out=dup[:], in0=cntp[:], scalar=cst[:, 280:281], in1=cnts[:],
                op0=Op.mult, op1=Op.add)
            wd = mp.tile([P, JJ], f32)
            nc.vector.tensor_scalar(wd[:], dup[:], 0.0, None, Op.is_equal)
            wv = mp.tile([P, JJ], f32)
            nc.vector.tensor_tensor(wv[:], wd[:], gtv[:], op=Op.mult)

            ec = mp.tile([P, 2 * JJ], f32)
            ec3 = ec[:].rearrange("p (j c) -> p j c", c=2)
            nc.scalar.activation(
                out=ec3, in_=cells3[:, :, 0:10:5], func=Act.Exp)
            scn = mp.tile([P, 2 * JJ], f32)
            nc.scalar.activation(out=scn[:], in_=ec[:], func=Act.Ln, bias=1.0)
            spc = mp.tile([P, JJ], f32)
            nc.vector.tensor_reduce(
                spc[:], scn[:].rearrange("p (j c) -> p j c", c=2),
                axis=AxX, op=Op.add)
            corrv = mp.tile([P, JJ], f32)
            nc.vector.scalar_tensor_tensor(
                out=corrv[:], in0=spc[:], scalar=1.0, in1=wv[:],
                op0=Op.mult, op1=Op.mult, accum_out=stats_dve[:, 3:4])

            if debug:
                dbg = mp.tile([P, 12 * JJ], f32)
                nc.vector.tensor_copy(out=dbg[:, 0:16], in_=sel[:])
                nc.vector.tensor_copy(out=dbg[:, 16:48], in_=bxy[:])
                nc.vector.tensor_copy(out=dbg[:, 48:80], in_=bwh[:])
                nc.vector.tensor_copy(out=dbg[:, 80:96], in_=btob[:])
                nc.vector.tensor_copy(out=dbg[:, 96:112], in_=coordt[:])
                nc.vector.tensor_copy(out=dbg[:, 112:128], in_=so[:])
                nc.vector.tensor_copy(out=dbg[:, 128:144], in_=rowf[:])
                nc.vector.tensor_copy(out=dbg[:, 144:176], in_=gji[:])
                nc.vector.tensor_copy(out=dbg[:, 176:192], in_=nll[:])
                nc.sync.dma_start(out=dbg_d[:], in_=dbg[:])

            # ---- cross-partition reduce: ones^T @ stats
            ps = pp.tile([1, 4 + NCHUNK], f32)
            nc.tensor.matmul(
                out=ps[:, 0:4], lhsT=cst[:, 281:282], rhs=stats_dve[:],
                start=True, stop=True)
            nc.tensor.matmul(
                out=ps[:, 4:4 + NCHUNK], lhsT=cst[:, 281:282], rhs=stats_act[:],
                start=True, stop=True)
            outt = mp.tile([1, 4 + NCHUNK], f32)
            nc.vector.tensor_copy(out=outt[:], in_=ps[:])
            nc.sync.dma_start(out=out_d[:], in_=outt[:])

    nc.compile()
    return nc


_CONSTS = _build_consts()
_NC_CACHE = {}


def _get_program(for_sim: bool = False) -> bass.Bass:
    key = bool(for_sim)
    if key not in _NC_CACHE:
        _NC_CACHE[key] = build_program(for_sim)
    return _NC_CACHE[key]


def make_in_maps(predictions, gt_boxes, gt_labels, gt_valid):
    predictions = np.ascontiguousarray(np.asarray(predictions), np.float32)
    gtb = np.ascontiguousarray(np.asarray(gt_boxes), np.float32)
    gtl = np.asarray(gt_labels).astype(np.float32)
    gtv = np.asarray(gt_valid).astype(np.float32)
    in_maps = []
    for c in range(NCORES):
        sl = slice(c * NIMG, (c + 1) * NIMG)
        in_maps.append({
            "pred": predictions[sl].reshape(ROWS, 18),
            "gtb": gtb[sl].reshape(NG, 4).reshape(P, JJ * 4),
            "gtl": gtl[sl].reshape(NG).reshape(P, JJ),
            "gtv": gtv[sl].reshape(NG).reshape(P, JJ),
            "consts": _CONSTS,
        })
    return in_maps


def combine_outputs(outs):
    """outs: list of (1, 12) per-core partials -> 5-tuple of f32 scalars."""
    t = np.stack([np.asarray(o).reshape(4 + NCHUNK) for o in outs]).astype(np.float64)
    s = t.sum(0)
    coord, obj, cls, corr = s[0], s[1], s[2], s[3]
    noobj = s[4:4 + NCHUNK].sum() - corr
    total = (LAMBDA_COORD * coord + obj + LAMBDA_NOOBJ * noobj + cls) / BATCH
    return (np.float32(total), np.float32(coord / BATCH),
            np.float32(obj / BATCH), np.float32(noobj / BATCH),
            np.float32(cls / BATCH))


def kernel(predictions, gt_boxes, gt_labels, gt_valid):
    from concourse.bass_utils import run_bass_kernel_spmd

    nc = _get_program(for_sim=False)
    in_maps = make_in_maps(predictions, gt_boxes, gt_labels, gt_valid)
    res = run_bass_kernel_spmd(nc, in_maps, list(range(NCORES))).results
    return combine_outputs([r["out"] for r in res])
